# revision 7
# baseline (speedup 1.0000x reference)
"""Trainium2 Bass kernel for CrossViewAttention.

Shards over (batch, query n-slice): core c handles b = c//4 and n positions
[s*256, (s+1)*256) with s = c%4, for all 3 views (768 query tokens). K/V are
computed per-core for the full batch row (no cross-core communication).

Hardcoded problem shape: features [3, 2, 256, 32, 32], DIM=256, 4 heads,
head_dim 64, L = 3*1024 = 3072 tokens per batch.
"""

import numpy as np
import ml_dtypes

import concourse.bass as bass
import concourse.tile as tile
from concourse import bacc, mybir
from concourse.bass_utils import run_bass_kernel_spmd
from concourse.masks import make_identity

F32 = mybir.dt.float32
BF16 = mybir.dt.bfloat16
NPBF16 = ml_dtypes.bfloat16

V, B, C, H, W = 3, 2, 256, 32, 32
N = H * W            # 1024
L = V * N            # 3072
NH, HD = 4, 64
SCALE = HD ** -0.5
LN_EPS = 1e-5
NCORES = 8
NSL = N // (NCORES // B)   # 256 n-positions per core
NQ = V * NSL               # 768 query tokens per core
JC = L // 128              # 24 key chunks
ITN = NQ // 256            # 3 i-chunks of 256
KB = 6                     # j-chunks per exp batch


def build_nc():
    nc = bacc.Bacc()

    xT = nc.dram_tensor("xT", [C, L], BF16, kind="ExternalInput")
    xQ = nc.dram_tensor("xQ", [C, NQ], BF16, kind="ExternalInput")
    wqT = nc.dram_tensor("wqT", [C, C], BF16, kind="ExternalInput")
    wkT = nc.dram_tensor("wkT", [C, C], BF16, kind="ExternalInput")
    wvT = nc.dram_tensor("wvT", [C, C], BF16, kind="ExternalInput")
    wpT = nc.dram_tensor("wpT", [C, C], BF16, kind="ExternalInput")
    bqT = nc.dram_tensor("bqT", [C, 1], F32, kind="ExternalInput")
    bkT = nc.dram_tensor("bkT", [C, 1], F32, kind="ExternalInput")
    bvR = nc.dram_tensor("bvR", [1, C], BF16, kind="ExternalInput")
    bpR = nc.dram_tensor("bpR", [1, C], F32, kind="ExternalInput")
    gaR = nc.dram_tensor("gaR", [1, C], F32, kind="ExternalInput")
    beR = nc.dram_tensor("beR", [1, C], F32, kind="ExternalInput")
    out = nc.dram_tensor("out", [C, NSL], F32, kind="ExternalOutput")

    def bcast_row(ap):
        # [1, C] dram row -> partition-stride-0 AP for DMA broadcast
        a = ap[:, :]
        return bass.AP(tensor=a.tensor, offset=a.offset, ap=[[0, 128], a.ap[1]])

    with tile.TileContext(nc) as tc:
        with tc.tile_pool(name="const", bufs=1) as cns, \
             tc.tile_pool(name="xp", bufs=1) as xp, \
             tc.tile_pool(name="kqv", bufs=1) as kqv, \
             tc.tile_pool(name="etp", bufs=2) as etp, \
             tc.tile_pool(name="sbc", bufs=2) as sbc, \
             tc.tile_pool(name="sbz", bufs=1) as sbz:

            # ---- constants / weights ----
            wq = [cns.tile([128, C], BF16, tag=f"wq{i}", name=f"wq{i}") for i in range(2)]
            wk = [cns.tile([128, C], BF16, tag=f"wk{i}", name=f"wk{i}") for i in range(2)]
            wv = [cns.tile([128, C], BF16, tag=f"wv{i}", name=f"wv{i}") for i in range(2)]
            wp = [cns.tile([64, C], BF16, tag=f"wp{i}", name=f"wp{i}") for i in range(NH)]
            for i in range(2):
                nc.sync.dma_start(out=wq[i], in_=wqT[i * 128:(i + 1) * 128, :])
                nc.sync.dma_start(out=wk[i], in_=wkT[i * 128:(i + 1) * 128, :])
                nc.sync.dma_start(out=wv[i], in_=wvT[i * 128:(i + 1) * 128, :])
            for h in range(NH):
                nc.sync.dma_start(out=wp[h], in_=wpT[h * 64:(h + 1) * 64, :])
            bq = [cns.tile([128, 1], F32, tag=f"bq{i}", name=f"bq{i}") for i in range(2)]
            bk = [cns.tile([128, 1], F32, tag=f"bk{i}", name=f"bk{i}") for i in range(2)]
            for i in range(2):
                nc.sync.dma_start(out=bq[i], in_=bqT[i * 128:(i + 1) * 128, :])
                nc.sync.dma_start(out=bk[i], in_=bkT[i * 128:(i + 1) * 128, :])
            bvr = cns.tile([1, C], BF16, tag="bvr", name="bvr")
            nc.sync.dma_start(out=bvr, in_=bvR[:, :])
            ones1 = cns.tile([1, 128], BF16, tag="ones1", name="ones1")
            nc.vector.memset(ones1, 1.0)
            bp_bc = cns.tile([128, C], F32, tag="bp_bc", name="bp_bc")
            ga_bc = cns.tile([128, C], F32, tag="ga_bc", name="ga_bc")
            be_bc = cns.tile([128, C], F32, tag="be_bc", name="be_bc")
            nc.sync.dma_start(out=bp_bc, in_=bcast_row(bpR))
            nc.sync.dma_start(out=ga_bc, in_=bcast_row(gaR))
            nc.sync.dma_start(out=be_bc, in_=bcast_row(beR))
            ident = cns.tile([128, 128], F32, tag="ident", name="ident")
            make_identity(nc, ident)
            eps_t = cns.tile([128, 1], F32, tag="eps_t", name="eps_t")
            nc.vector.memset(eps_t, LN_EPS)

            xt = [xp.tile([128, L], BF16, tag=f"xt{i}", name=f"xt{i}") for i in range(2)]
            xq = [xp.tile([128, NQ], BF16, tag=f"xq{i}", name=f"xq{i}") for i in range(2)]
            for i in range(2):
                nc.sync.dma_start(out=xt[i], in_=xT[i * 128:(i + 1) * 128, :])
                nc.sync.dma_start(out=xq[i], in_=xQ[i * 128:(i + 1) * 128, :])

            # persistent per-head tensors
            kT = [kqv.tile([128, L], BF16, tag=f"kT{i}", name=f"kT{i}") for i in range(2)]
            qT = [kqv.tile([128, NQ], BF16, tag=f"qT{i}", name=f"qT{i}") for i in range(2)]
            vaug = kqv.tile([128, JC, NH, HD + 1], BF16, tag="vaug", name="vaug")
            oTs = [kqv.tile([HD + 1, NQ], BF16, tag=f"oTs{h}", name=f"oTs{h}") for h in range(NH)]
            rs = kqv.tile([HD + 1, NH, NQ], F32, tag="rs", name="rs")

            nc.vector.memset(vaug[:, :, :, HD:HD + 1], 1.0)

            # ---- phase A: projections ----
            with tc.tile_pool(name="psA", bufs=2, space="PSUM") as psA:
                # kT[dt] rows = heads (2dt, 2dt+1); kT[d, j] = sum_c WkT[c, d] x[j, c]
                for dt in range(2):
                    for nb in range(L // 512):
                        kp = psA.tile([128, 512], F32, tag="kp", name="kp")
                        for ct in range(2):
                            nc.tensor.matmul(
                                kp, wk[ct][:, dt * 128:(dt + 1) * 128],
                                xt[ct][:, nb * 512:(nb + 1) * 512],
                                start=(ct == 0), stop=(ct == 1))
                        nc.vector.tensor_scalar_add(
                            out=kT[dt][:, nb * 512:(nb + 1) * 512], in0=kp,
                            scalar1=bk[dt])
                # v[j, d] all heads + bias via K=1 ones matmul
                for jc in range(JC):
                    vp = psA.tile([128, C], F32, tag="vp", name="vp")
                    nc.tensor.matmul(vp, xt[0][:, jc * 128:(jc + 1) * 128], wv[0],
                                     start=True, stop=False)
                    nc.tensor.matmul(vp, xt[1][:, jc * 128:(jc + 1) * 128], wv[1],
                                     start=False, stop=False)
                    nc.tensor.matmul(vp, ones1, bvr, start=False, stop=True)
                    nc.vector.tensor_copy(
                        out=vaug[:, jc, :, 0:HD],
                        in_=vp[:, :].rearrange("p (h e) -> p h e", h=NH))
                # qT
                for dt in range(2):
                    qp = psA.tile([128, 3, 256], F32, tag="qp", name="qp")
                    for vv in range(3):
                        for ct in range(2):
                            nc.tensor.matmul(
                                qp[:, vv], wq[ct][:, dt * 128:(dt + 1) * 128],
                                xq[ct][:, vv * 256:(vv + 1) * 256],
                                start=(ct == 0), stop=(ct == 1))
                    nc.vector.tensor_scalar_add(
                        out=qT[dt][:, :].rearrange("p (a b) -> p a b", a=3),
                        in0=qp, scalar1=bq[dt])

            # ---- phase B: attention per head ----
            with tc.tile_pool(name="psB", bufs=2, space="PSUM") as psB, \
                 tc.tile_pool(name="psB1", bufs=1, space="PSUM") as psB1:
                for h in range(NH):
                    t_, r0 = h // 2, (h % 2) * 64
                    for it in range(ITN):
                        ot = psB1.tile([HD + 1, 256], F32, tag="ot", name="ot")
                        for bb in range(JC // KB):
                            sp = psB.tile([128, KB, 256], F32, tag="sp", name="sp")
                            et = etp.tile([128, KB, 256], BF16, tag="et", name="et")
                            for jj in range(KB):
                                jc = bb * KB + jj
                                nc.tensor.matmul(
                                    sp[:, jj],
                                    kT[t_][r0:r0 + 64, jc * 128:(jc + 1) * 128],
                                    qT[t_][r0:r0 + 64, it * 256:(it + 1) * 256],
                                    start=True, stop=True)
                            nc.scalar.activation(
                                out=et, in_=sp,
                                func=mybir.ActivationFunctionType.Exp,
                                scale=SCALE)
                            for jj in range(KB):
                                jc = bb * KB + jj
                                nc.tensor.matmul(
                                    ot, vaug[:, jc, h, :], et[:, jj],
                                    start=(bb == 0 and jj == 0),
                                    stop=(bb == JC // KB - 1 and jj == KB - 1))
                        nc.vector.tensor_copy(
                            out=oTs[h][:, it * 256:(it + 1) * 256], in_=ot)
                        nc.vector.tensor_copy(
                            out=rs[HD:HD + 1, h, it * 256:(it + 1) * 256],
                            in_=ot[HD:HD + 1, :])

            # ---- phase C: rowsum recip, out proj, LN, view mean ----
            with tc.tile_pool(name="psC", bufs=1, space="PSUM") as psC, \
                 tc.tile_pool(name="psT", bufs=2, space="PSUM") as psT:
                rrp = psC.tile([128, NH, 6], F32, tag="rrp", name="rrp")
                for hh in range(NH):
                    for ic in range(6):
                        nc.tensor.transpose(
                            rrp[:, hh, ic:ic + 1],
                            rs[HD:HD + 1, hh, ic * 128:(ic + 1) * 128],
                            ident[HD:HD + 1, HD:HD + 1])
                rrT = sbz.tile([128, NH, 6], F32, tag="rrT", name="rrT")
                nc.vector.reciprocal(out=rrT, in_=rrp)

                zf = [sbz.tile([128, C], F32, tag=f"zf{ic}", name=f"zf{ic}") for ic in range(6)]
                yp = [psC.tile([128, C], F32, tag=f"yp{h}", name=f"yp{h}") for h in range(NH)]
                for ic in range(6):
                    for h in range(NH):
                        nc.tensor.matmul(
                            yp[h], oTs[h][0:HD, ic * 128:(ic + 1) * 128],
                            wp[h], start=True, stop=True)
                    t0 = sbc.tile([128, C], F32, tag="t0", name="t0")
                    t1 = sbc.tile([128, C], F32, tag="t1", name="t1")
                    t2 = sbc.tile([128, C], F32, tag="t2", name="t2")
                    t3 = sbc.tile([128, C], F32, tag="t3", name="t3")
                    nc.scalar.activation(out=t0, in_=yp[0],
                                         func=mybir.ActivationFunctionType.Copy,
                                         scale=rrT[:, 0, ic:ic + 1])
                    nc.scalar.activation(out=t1, in_=yp[1],
                                         func=mybir.ActivationFunctionType.Copy,
                                         scale=rrT[:, 1, ic:ic + 1])
                    nc.vector.tensor_scalar_mul(out=t2, in0=yp[2],
                                                scalar1=rrT[:, 2, ic:ic + 1])
                    nc.vector.tensor_scalar_mul(out=t3, in0=yp[3],
                                                scalar1=rrT[:, 3, ic:ic + 1])
                    a0 = sbc.tile([128, C], F32, tag="a0", name="a0")
                    a1 = sbc.tile([128, C], F32, tag="a1", name="a1")
                    nc.vector.tensor_add(out=a0, in0=t0, in1=t1)
                    nc.vector.tensor_add(out=a1, in0=t2, in1=t3)
                    nc.vector.tensor_add(out=a0, in0=a0, in1=a1)
                    z = sbc.tile([128, C], F32, tag="z", name="z")
                    nc.vector.tensor_add(out=z, in0=a0, in1=bp_bc)
                    # LayerNorm over C
                    st = sbc.tile([128, 6], F32, tag="st", name="st")
                    mv = sbc.tile([128, 2], F32, tag="mv", name="mv")
                    nc.vector.bn_stats(out=st, in_=z)
                    nc.vector.bn_aggr(out=mv, in_=st)
                    sd = sbc.tile([128, 1], F32, tag="sd", name="sd")
                    nc.scalar.activation(out=sd, in_=mv[:, 1:2],
                                         func=mybir.ActivationFunctionType.Sqrt,
                                         bias=eps_t)
                    rstd = sbc.tile([128, 1], F32, tag="rstd", name="rstd")
                    nc.vector.reciprocal(out=rstd, in_=sd)
                    zn = sbc.tile([128, C], F32, tag="zn", name="zn")
                    nc.vector.tensor_scalar(out=zn, in0=z, scalar1=mv[:, 0:1],
                                            scalar2=rstd,
                                            op0=mybir.AluOpType.subtract,
                                            op1=mybir.AluOpType.mult)
                    nc.vector.tensor_mul(out=zn, in0=zn, in1=ga_bc)
                    nc.vector.tensor_add(out=zf[ic], in0=zn, in1=be_bc)

                # view mean + transpose to [C, n]
                osb = [sbz.tile([128, NSL], F32, tag=f"osb{i}", name=f"osb{i}") for i in range(2)]
                for nk in range(2):
                    u = sbc.tile([128, C], F32, tag="u", name="u")
                    nc.vector.tensor_add(out=u, in0=zf[nk], in1=zf[2 + nk])
                    nc.vector.tensor_add(out=u, in0=u, in1=zf[4 + nk])
                    nc.vector.tensor_scalar_mul(out=u, in0=u, scalar1=1.0 / 3.0)
                    for cc in range(2):
                        tp = psT.tile([128, 128], F32, tag="tp", name="tp")
                        nc.tensor.transpose(tp, u[:, cc * 128:(cc + 1) * 128],
                                            ident)
                        nc.vector.tensor_copy(
                            out=osb[cc][:, nk * 128:(nk + 1) * 128], in_=tp)
                for cc in range(2):
                    nc.sync.dma_start(out=out[cc * 128:(cc + 1) * 128, :],
                                      in_=osb[cc])

    nc.finalize()
    return nc


_NC = None


def _get_nc():
    global _NC
    if _NC is None:
        _NC = build_nc()
    return _NC


def make_in_maps(inputs):
    f = np.asarray(inputs["features"], np.float32)
    x = f.reshape(V, B, C, N).transpose(1, 0, 3, 2).reshape(B, L, C)
    wq = np.ascontiguousarray(np.asarray(inputs["Wq"], np.float32).T).astype(NPBF16)
    wk = np.ascontiguousarray(np.asarray(inputs["Wk"], np.float32).T).astype(NPBF16)
    wv = np.ascontiguousarray(np.asarray(inputs["Wv"], np.float32).T).astype(NPBF16)
    wp = np.ascontiguousarray(np.asarray(inputs["Wp"], np.float32).T).astype(NPBF16)
    bq = np.asarray(inputs["bq"], np.float32).reshape(C, 1)
    bk = np.asarray(inputs["bk"], np.float32).reshape(C, 1)
    bv = np.asarray(inputs["bv"], np.float32).reshape(1, C).astype(NPBF16)
    bp = np.asarray(inputs["bp"], np.float32).reshape(1, C)
    ga = np.asarray(inputs["gamma"], np.float32).reshape(1, C)
    be = np.asarray(inputs["beta"], np.float32).reshape(1, C)
    in_maps = []
    for c in range(NCORES):
        b, s = c // (NCORES // B), c % (NCORES // B)
        xTb = np.ascontiguousarray(x[b].T).astype(NPBF16)
        qtok = np.concatenate(
            [np.arange(v * N + s * NSL, v * N + s * NSL + NSL) for v in range(V)])
        in_maps.append({
            "xT": xTb, "xQ": np.ascontiguousarray(xTb[:, qtok]),
            "wqT": wq, "wkT": wk, "wvT": wv, "wpT": wp,
            "bqT": bq, "bkT": bk, "bvR": bv, "bpR": bp, "gaR": ga, "beR": be,
        })
    return in_maps


def assemble(results):
    out = np.empty((B, C, N), np.float32)
    for c in range(NCORES):
        b, s = c // (NCORES // B), c % (NCORES // B)
        out[b, :, s * NSL:(s + 1) * NSL] = results[c]["out"]
    return out.reshape(B, C, H, W)


def kernel(**inputs):
    nc = _get_nc()
    res = run_bass_kernel_spmd(nc, make_in_maps(inputs),
                               core_ids=list(range(NCORES)))
    return assemble(res.results)
